# revision 1
# baseline (speedup 1.0000x reference)
import numpy as np

# GRIT attention: B=4, N=512, C=128, E=32, A=8 (shapes hardcoded per contract).
# Sharding: 8 shards = batch (4) x query-node half (2). Softmax/sum reductions
# run over the second N axis, so every reduction is device-local.
B, N, C, E, A = 4, 512, 128, 32, 8
E2 = 2 * E
NS = N // 2  # query rows per shard


def _math(jnp, qnode, node, edge, embedding, Wq, bq, Wk, bk, Wv, bv, Wew, Web,
          Wen, ben, Wa, Wno, bno, Weo):
    def rho(x):
        return jnp.sign(x) * jnp.sqrt(jnp.abs(x))

    def gelu(x):
        from jax.scipy.special import erf
        return x * 0.5 * (1.0 + erf(x / np.sqrt(2.0).astype(np.float32)))

    Q = (qnode @ Wq.T + bq).reshape(NS, A, E2)
    K = (node @ Wk.T + bk).reshape(N, A, E2)
    QK = jnp.einsum("iae,jae->ije", Q, K)
    qk_w, qk_b = QK[..., :E], QK[..., E:]
    edge_new = gelu(rho(qk_w * (edge @ Wew.T)) + edge @ Web.T + qk_b + embedding)
    logits = jnp.einsum("ije,e->ij", edge_new, Wa[0])
    m = jnp.max(logits, axis=-1, keepdims=True)
    ex = jnp.exp(logits - m)
    alpha = ex / jnp.sum(ex, axis=-1, keepdims=True)
    V = node @ Wv.T + bv
    agg = jnp.einsum("ije,ij->ie", edge_new, alpha)
    node_out = jnp.einsum("ij,jc->ic", alpha, V) + agg @ Wen.T + ben
    node_out = node_out @ Wno.T + bno
    edge_out = edge_new @ Weo.T
    return node_out, edge_out


def _run_pmap(inputs):
    import jax
    import jax.numpy as jnp

    devs = jax.devices()
    assert len(devs) >= 8, f"need 8 devices, got {len(devs)}"

    node = inputs["node"]
    edge = inputs["edge"]
    emb = inputs["embedding"]
    wnames = ["Wq", "bq", "Wk", "bk", "Wv", "bv", "Wew", "Web", "Wen", "ben",
              "Wa", "Wno", "bno", "Weo"]

    # shard s = b * 2 + h: batch b, query rows [h*NS, (h+1)*NS)
    qnode_s = np.stack([node[s // 2, (s % 2) * NS:(s % 2 + 1) * NS] for s in range(8)])
    node_s = np.stack([node[s // 2] for s in range(8)])
    edge_s = np.stack([edge[s // 2, (s % 2) * NS:(s % 2 + 1) * NS] for s in range(8)])
    emb_s = np.stack([emb[s // 2, (s % 2) * NS:(s % 2 + 1) * NS] for s in range(8)])
    w_s = [np.broadcast_to(inputs[w], (8,) + inputs[w].shape) for w in wnames]

    f = jax.pmap(lambda qn, nd, ed, em, *ws: _math(jnp, qn, nd, ed, em, *ws),
                 devices=devs[:8])
    no_s, eo_s = f(qnode_s, node_s, edge_s, emb_s, *w_s)
    no_s = np.asarray(no_s)
    eo_s = np.asarray(eo_s)

    node_out = no_s.reshape(B, 2 * NS, C).astype(np.float32)
    edge_out = eo_s.reshape(B, N, N, E).astype(np.float32)
    return node_out, edge_out


def _run_cpu(inputs):
    import jax
    import jax.numpy as jnp
    node = inputs["node"]
    outs_n, outs_e = [], []
    args = [inputs[w] for w in ["Wq", "bq", "Wk", "bk", "Wv", "bv", "Wew", "Web",
                                "Wen", "ben", "Wa", "Wno", "bno", "Weo"]]
    f = jax.jit(lambda qn, nd, ed, em, *ws: _math(jnp, qn, nd, ed, em, *ws),
                backend="cpu")
    for b in range(B):
        ns, es = [], []
        for h in range(2):
            no, eo = f(node[b, h * NS:(h + 1) * NS], node[b],
                       inputs["edge"][b, h * NS:(h + 1) * NS],
                       inputs["embedding"][b, h * NS:(h + 1) * NS], *args)
            ns.append(np.asarray(no))
            es.append(np.asarray(eo))
        outs_n.append(np.concatenate(ns, 0))
        outs_e.append(np.concatenate(es, 0))
    return (np.stack(outs_n).astype(np.float32),
            np.stack(outs_e).astype(np.float32))


def kernel(**inputs):
    try:
        return _run_pmap(inputs)
    except Exception:
        return _run_cpu(inputs)


# revision 2
# speedup vs baseline: 1.0181x; 1.0181x over previous
import numpy as np

# GRIT attention: B=4, N=512, C=128, E=32, A=8 (shapes hardcoded per contract).
# Sharding: 8 shards = batch (4) x query-node half (2). Softmax/sum reductions
# run over the second N axis, so every reduction is device-local.
B, N, C, E, A = 4, 512, 128, 32, 8
E2 = 2 * E
NS = N // 2  # query rows per shard


def _math(jnp, qnode, node, edge, embedding, Wq, bq, Wk, bk, Wv, bv, Wew, Web,
          Wen, ben, Wa, Wno, bno, Weo):
    def rho(x):
        return jnp.sign(x) * jnp.sqrt(jnp.abs(x))

    def gelu(x):
        from jax.scipy.special import erf
        return x * 0.5 * (1.0 + erf(x / np.sqrt(2.0).astype(np.float32)))

    Q = (qnode @ Wq.T + bq).reshape(NS, A, E2)
    K = (node @ Wk.T + bk).reshape(N, A, E2)
    QK = jnp.einsum("iae,jae->ije", Q, K)
    qk_w, qk_b = QK[..., :E], QK[..., E:]
    edge_new = gelu(rho(qk_w * (edge @ Wew.T)) + edge @ Web.T + qk_b + embedding)
    logits = jnp.einsum("ije,e->ij", edge_new, Wa[0])
    m = jnp.max(logits, axis=-1, keepdims=True)
    ex = jnp.exp(logits - m)
    alpha = ex / jnp.sum(ex, axis=-1, keepdims=True)
    V = node @ Wv.T + bv
    agg = jnp.einsum("ije,ij->ie", edge_new, alpha)
    node_out = jnp.einsum("ij,jc->ic", alpha, V) + agg @ Wen.T + ben
    node_out = node_out @ Wno.T + bno
    edge_out = edge_new @ Weo.T
    return node_out, edge_out


def _run_pmap(inputs):
    import jax
    import jax.numpy as jnp

    devs = jax.devices()
    assert len(devs) >= 8, f"need 8 devices, got {len(devs)}"

    node = inputs["node"]
    edge = inputs["edge"]
    emb = inputs["embedding"]
    wnames = ["Wq", "bq", "Wk", "bk", "Wv", "bv", "Wew", "Web", "Wen", "ben",
              "Wa", "Wno", "bno", "Weo"]

    # shard s = b * 2 + h: batch b, query rows [h*NS, (h+1)*NS).
    # Contiguous inputs make these zero-copy reshapes.
    qnode_s = np.ascontiguousarray(node).reshape(8, NS, C)
    node_s = np.repeat(node, 2, axis=0)
    edge_s = np.ascontiguousarray(edge).reshape(8, NS, N, E)
    emb_s = np.ascontiguousarray(emb).reshape(8, NS, N, E)
    w_s = [np.broadcast_to(inputs[w], (8,) + inputs[w].shape) for w in wnames]

    f = jax.pmap(lambda qn, nd, ed, em, *ws: _math(jnp, qn, nd, ed, em, *ws),
                 devices=devs[:8])
    no_s, eo_s = f(qnode_s, node_s, edge_s, emb_s, *w_s)
    no_s = np.asarray(no_s)
    eo_s = np.asarray(eo_s)

    node_out = no_s.reshape(B, 2 * NS, C).astype(np.float32)
    edge_out = eo_s.reshape(B, N, N, E).astype(np.float32)
    return node_out, edge_out


def _run_cpu(inputs):
    import jax
    import jax.numpy as jnp
    node = inputs["node"]
    outs_n, outs_e = [], []
    args = [inputs[w] for w in ["Wq", "bq", "Wk", "bk", "Wv", "bv", "Wew", "Web",
                                "Wen", "ben", "Wa", "Wno", "bno", "Weo"]]
    f = jax.jit(lambda qn, nd, ed, em, *ws: _math(jnp, qn, nd, ed, em, *ws),
                backend="cpu")
    for b in range(B):
        ns, es = [], []
        for h in range(2):
            no, eo = f(node[b, h * NS:(h + 1) * NS], node[b],
                       inputs["edge"][b, h * NS:(h + 1) * NS],
                       inputs["embedding"][b, h * NS:(h + 1) * NS], *args)
            ns.append(np.asarray(no))
            es.append(np.asarray(eo))
        outs_n.append(np.concatenate(ns, 0))
        outs_e.append(np.concatenate(es, 0))
    return (np.stack(outs_n).astype(np.float32),
            np.stack(outs_e).astype(np.float32))


def kernel(**inputs):
    try:
        return _run_pmap(inputs)
    except Exception:
        return _run_cpu(inputs)
